# revision 5
# baseline (speedup 1.0000x reference)
"""Trainium2 Bass kernel for nn_BinarySquareClassifier (3-layer LIF SNN).

Strategy (pure data parallel over batch, 8 cores, B=2048 -> 256/core):
- One stacked f32 matmul per time-chunk computes h1/h2/h3 for all three
  layers at once: lhsT [126, 98] = blockdiag(W1.T, W2.T, W3.T); layer l's
  input rows come one chunk later than layer l-1's outputs (pipeline skew),
  so the serial LIF scans of the three layers run on time-shifted frames
  and stack into one [98, 256] membrane state M.
- Biases are folded away with the shift m^ = m - b/(1-beta): per-partition
  spike thresholds th = 1 - b/(1-beta), zero-input warmup freezing via +BIG
  thresholds and pre-decayed initial states.
- Per scan step (DVE): r = (M > th) [doubles as the spike written into the
  next chunk's matmul rhs and as the reset tensor], u = beta*M + H (PSUM),
  M = u - r.
- Layer-3 spike rows (rhs rows 126:128) are stashed to a [128, 4096] SBUF
  tile via SBUF->SBUF DMA (partition = time) and reduced at the end with
  tensor adds + a ones-vector matmul over partitions.
"""

import numpy as np
from contextlib import ExitStack

B_FULL, C_IN, T_FULL = 2048, 30, 1024
N_CORES = 8
B = B_FULL // N_CORES           # 256 batch per core
TC = 8                          # timesteps per chunk
N_CHUNKS = T_FULL // TC         # 128
BETA = 0.9
BIG = 3.0e38

_cache = {}


def _build_program():
    import concourse.bass as bass
    import concourse.mybir as mybir
    import concourse.tile as tile

    nc = bass.Bass("TRN2", target_bir_lowering=False, debug=False,
                   num_devices=N_CORES)
    dt = mybir.dt.float32
    AOT = mybir.AluOpType

    xt_in = nc.dram_tensor("xt", [C_IN, T_FULL, B], dt, kind="ExternalInput").ap()
    lhsT_in = nc.dram_tensor("lhsT", [128, 98], dt, kind="ExternalInput").ap()
    th_in = nc.dram_tensor("th", [98, 3], dt, kind="ExternalInput").ap()
    m0_in = nc.dram_tensor("m0", [98, B], dt, kind="ExternalInput").ap()
    acc_out = nc.dram_tensor("acc", [1, 512], dt, kind="ExternalOutput").ap()

    with ExitStack() as ctx:
        tc = ctx.enter_context(tile.TileContext(nc))
        pool = ctx.enter_context(tc.tile_pool(name="sb", bufs=1))
        psum_pool = ctx.enter_context(tc.tile_pool(name="ps", bufs=1, space="PSUM"))

        t_lhsT = pool.tile([128, 98], dt, tag="lhsT", name="lhsT")
        t_th = pool.tile([98, 3], dt, tag="th", name="th")
        t_M = pool.tile([98, B], dt, tag="M", name="M")
        t_u = pool.tile([98, B], dt, tag="u", name="u")
        t_rhs = [pool.tile([128, TC * B], dt, tag=f"rhs{i}", name=f"rhs{i}") for i in range(2)]
        t_stash = pool.tile([128, 4096], dt, tag="stash", name="stash")
        t_ones = pool.tile([128, 1], dt, tag="ones", name="ones")
        t_part = pool.tile([128, 512], dt, tag="part", name="part")
        t_accf = pool.tile([1, 512], dt, tag="accf", name="accf")
        t_ps = [psum_pool.tile([98, TC * B], dt, tag=f"H{i}", name=f"H{i}") for i in range(2)]
        t_psr = t_ps[1][0:1, 0:512]  # reuse a PSUM bank for the final reduce

        nc.gpsimd.dma_start(out=t_lhsT[:], in_=lhsT_in[:])
        nc.gpsimd.dma_start(out=t_th[:], in_=th_in[:])
        nc.gpsimd.dma_start(out=t_M[:], in_=m0_in[:])
        nc.gpsimd.memset(t_ones[:], 1.0)
        # zero the spike rows of both rhs buffers and the stash
        for r in t_rhs:
            nc.gpsimd.memset(r[:], 0.0)

        # x prefetch for chunk 0
        nc.gpsimd.dma_start(
            out=t_rhs[0][98:128, :], in_=xt_in[:, 0:TC, :].rearrange("c t b -> c (t b)")
        )

        def th_col(c):
            return 0 if c == 0 else (1 if c == 1 else 2)

        for c in range(N_CHUNKS + 2):
            cur = t_rhs[c % 2]
            nxt = t_rhs[(c + 1) % 2]
            ps = t_ps[c % 2]
            th = t_th[:, th_col(c):th_col(c) + 1]

            # prefetch x for chunk c+1 (overlaps this chunk's scan)
            if c + 1 < N_CHUNKS:
                nc.gpsimd.dma_start(
                    out=nxt[98:128, :],
                    in_=xt_in[:, (c + 1) * TC:(c + 2) * TC, :].rearrange(
                        "c t b -> c (t b)"),
                )

            # stacked matmul for this chunk, 4 sub-matmuls of 512 columns
            for blk in range(4):
                sl = slice(blk * 512, (blk + 1) * 512)
                nc.tensor.matmul(ps[:, sl], t_lhsT[:], cur[0:128, sl])

            # serial LIF scan over the TC steps
            for tau in range(TC):
                if tau == 0:
                    r_ap = cur[0:98, (TC - 1) * B:TC * B]
                else:
                    r_ap = nxt[0:98, (tau - 1) * B:tau * B]
                    nc.vector.tensor_scalar(r_ap, t_M[:], th, None, AOT.is_gt)
                nc.vector.scalar_tensor_tensor(
                    t_u[:], t_M[:], BETA, ps[:, tau * B:(tau + 1) * B],
                    AOT.mult, AOT.add)
                nc.vector.tensor_tensor(t_M[:], t_u[:], r_ap, AOT.subtract)

            # boundary spike (local time TC-1) -> last column of nxt
            nc.vector.tensor_scalar(
                nxt[0:98, (TC - 1) * B:TC * B], t_M[:], th, None, AOT.is_gt)

            # stash layer-3 spike rows for frame f = c-2
            f = c - 2
            if 0 <= f < N_CHUNKS:
                p0 = (f % 16) * TC
                cb = f // 16
                for j in range(2):
                    nc.gpsimd.dma_start(
                        out=t_stash[p0:p0 + TC,
                                    cb * 512 + j * B:cb * 512 + (j + 1) * B],
                        in_=nxt[96 + j:97 + j, :],
                    )

        # reduce stash: sum the 8 column blocks, then sum over partitions
        nc.vector.tensor_tensor(
            t_part[:], t_stash[:, 0:512], t_stash[:, 512:1024], AOT.add)
        for cb in range(2, 8):
            nc.vector.tensor_tensor(
                t_part[:], t_part[:], t_stash[:, cb * 512:(cb + 1) * 512], AOT.add)
        nc.tensor.matmul(t_psr[:], t_ones[:], t_part[:])
        nc.scalar.copy(t_accf[:], t_psr[:])
        nc.gpsimd.dma_start(out=acc_out[:], in_=t_accf[:])

    from waitsplit import split_multi_waits
    split_multi_waits(nc)
    return nc


def _host_consts(W1, b1, W2, b2, W3, b3):
    lhsT = np.zeros((128, 98), np.float32)
    lhsT[98:128, 0:64] = W1.T
    lhsT[0:64, 64:96] = W2.T
    lhsT[64:96, 96:98] = W3.T
    c1 = (b1.astype(np.float64) / (1.0 - BETA)).astype(np.float32)
    c2 = (b2.astype(np.float64) / (1.0 - BETA)).astype(np.float32)
    c3 = (b3.astype(np.float64) / (1.0 - BETA)).astype(np.float32)
    th_main = np.concatenate([
        (1.0 - c1.astype(np.float64)).astype(np.float32),
        (1.0 - c2.astype(np.float64)).astype(np.float32),
        (1.0 - c3.astype(np.float64)).astype(np.float32),
    ]).astype(np.float32)
    th_w0 = th_main.copy()
    th_w0[64:98] = BIG
    th_w1 = th_main.copy()
    th_w1[96:98] = BIG
    th = np.stack([th_w0, th_w1, th_main], axis=1)  # [98, 3]
    beta64 = np.float64(np.float32(BETA))
    m0 = np.zeros(98, np.float32)
    m0[0:64] = -c1
    m0[64:96] = (-c2.astype(np.float64) / beta64 ** TC).astype(np.float32)
    m0[96:98] = (-c3.astype(np.float64) / beta64 ** (2 * TC)).astype(np.float32)
    m0b = np.ascontiguousarray(
        np.broadcast_to(m0[:, None], (98, B))).astype(np.float32)
    return lhsT, th, m0b


def kernel(x, W1, b1, W2, b2, W3, b3):
    from concourse.bass_utils import run_bass_kernel_spmd

    x = np.asarray(x, np.float32)
    W1 = np.asarray(W1, np.float32); b1 = np.asarray(b1, np.float32)
    W2 = np.asarray(W2, np.float32); b2 = np.asarray(b2, np.float32)
    W3 = np.asarray(W3, np.float32); b3 = np.asarray(b3, np.float32)

    if "nc" not in _cache:
        _cache["nc"] = _build_program()
    nc = _cache["nc"]

    lhsT, th, m0b = _host_consts(W1, b1, W2, b2, W3, b3)
    in_maps = []
    for core in range(N_CORES):
        xs = x[core * B:(core + 1) * B]                # [256, 30, 1024]
        xt = np.ascontiguousarray(np.transpose(xs, (1, 2, 0)))  # [30, 1024, 256]
        in_maps.append({"xt": xt, "lhsT": lhsT, "th": th, "m0": m0b})

    res = run_bass_kernel_spmd(nc, in_maps, list(range(N_CORES)))
    out = np.empty((B_FULL, 2), np.float32)
    for core in range(N_CORES):
        a = res.results[core]["acc"].reshape(2, 256)   # [j, b]
        out[core * B:(core + 1) * B] = a.T
    return out


# revision 14
# speedup vs baseline: 6357.6127x; 6357.6127x over previous
"""Trainium2 Bass kernel for nn_BinarySquareClassifier (3-layer LIF SNN).

Strategy (pure data parallel over batch, 8 cores, B=2048 -> 256/core):
- One stacked f32 matmul per time-chunk computes h1/h2/h3 for all three
  layers at once: lhsT [126, 98] = blockdiag(W1.T, W2.T, W3.T); layer l's
  input rows come one chunk later than layer l-1's outputs (pipeline skew),
  so the serial LIF scans of the three layers run on time-shifted frames
  and stack into one [98, 256] membrane state M.
- Biases are folded away with the shift m^ = m - b/(1-beta): per-partition
  spike thresholds th = 1 - b/(1-beta), zero-input warmup freezing via +BIG
  thresholds and pre-decayed initial states.
- Per scan step, software-pipelined over two batch halves on DVE
  (u = beta*M + H then M = u - r, the halves' ops interleaved to cover
  semaphore latency); the spike op r = (M > th) runs on the Pool engine
  and doubles as the spike row written into the next chunk's matmul rhs.
  The ACT engine copies each PSUM matmul block to SBUF for cheaper reads.
- Layer-3 spike rows (rhs rows 96:98) are stashed to a [128, 4096] SBUF
  tile via SBUF->SBUF DMA (partition = time) and reduced at the end with
  tensor adds + a ones-vector matmul over partitions.
"""

import numpy as np
from contextlib import ExitStack

B_FULL, C_IN, T_FULL = 2048, 30, 1024
N_CORES = 8
B = B_FULL // N_CORES           # 256 batch per core
TC = 8                          # timesteps per chunk
N_CHUNKS = T_FULL // TC         # 128
BETA = 0.9
BIG = 3.0e38

_cache = {}


def _split_multi_waits(nc):
    """This container's walrus accepts only ONE sync-wait per instruction;
    hoist extra waits onto same-engine NoOps inserted just before."""
    import concourse.mybir as mybir
    counter = 0
    for f in nc.m.functions:
        for blk in f.blocks:
            out = []
            changed = False
            for inst in blk.instructions:
                si = inst.sync_info
                if si is not None and si.on_wait is not None and len(si.on_wait) > 1:
                    waits = list(si.on_wait)
                    for w in waits[:-1]:
                        counter += 1
                        nop = mybir.InstNoOp(
                            name=f"waitsplit-{counter}", ins=[], outs=[])
                        nop.engine = inst.engine
                        nop.sync_info = mybir.SyncInfo(on_wait=[w], on_update=[])
                        out.append(nop)
                    inst.sync_info = mybir.SyncInfo(
                        on_wait=[waits[-1]], on_update=list(si.on_update or []))
                    changed = True
                out.append(inst)
            if changed:
                try:
                    blk.instructions[:] = out
                except TypeError:
                    blk.instructions = out


def _build_program(SP=B):
    import concourse.bass as bass
    import concourse.mybir as mybir
    import concourse.tile as tile

    nc = bass.Bass("TRN2", target_bir_lowering=False, debug=False,
                   num_devices=N_CORES)
    dt = mybir.dt.float32
    AOT = mybir.AluOpType

    xt_in = nc.dram_tensor("xt", [C_IN, T_FULL, B], dt, kind="ExternalInput").ap()
    lhsT_in = nc.dram_tensor("lhsT", [128, 98], dt, kind="ExternalInput").ap()
    th_in = nc.dram_tensor("th", [98, 3], dt, kind="ExternalInput").ap()
    m0_in = nc.dram_tensor("m0", [98, B], dt, kind="ExternalInput").ap()
    acc_out = nc.dram_tensor("acc", [1, 512], dt, kind="ExternalOutput").ap()

    with ExitStack() as ctx:
        tc = ctx.enter_context(tile.TileContext(nc))
        pool = ctx.enter_context(tc.tile_pool(name="sb", bufs=1))
        psum_pool = ctx.enter_context(tc.tile_pool(name="ps", bufs=1, space="PSUM"))

        t_lhsT = pool.tile([128, 98], dt, tag="lhsT", name="lhsT")
        t_th = pool.tile([98, 3], dt, tag="th", name="th")
        t_M = pool.tile([98, B], dt, tag="M", name="M")
        t_u = pool.tile([98, B], dt, tag="u", name="u")
        t_rhs = [pool.tile([128, TC * B], dt, tag=f"rhs{i}", name=f"rhs{i}") for i in range(2)]
        t_stash = pool.tile([128, 4096], dt, tag="stash", name="stash")
        t_ones = pool.tile([128, 1], dt, tag="ones", name="ones")
        t_part = pool.tile([128, 512], dt, tag="part", name="part")
        t_accf = pool.tile([1, 512], dt, tag="accf", name="accf")
        t_ps = [psum_pool.tile([98, TC * B], dt, tag=f"H{i}", name=f"H{i}") for i in range(2)]
        t_hs = [pool.tile([98, TC * B], dt, tag=f"Hs{i}", name=f"Hs{i}") for i in range(2)]
        t_psr = t_ps[1][0:1, 0:512]  # reuse a PSUM bank for the final reduce

        nc.gpsimd.dma_start(out=t_lhsT[:], in_=lhsT_in[:])
        nc.gpsimd.dma_start(out=t_th[:], in_=th_in[:])
        nc.gpsimd.dma_start(out=t_M[:], in_=m0_in[:])
        nc.gpsimd.memset(t_ones[:], 1.0)
        # zero the spike rows of both rhs buffers and the stash
        for r in t_rhs:
            nc.gpsimd.memset(r[:], 0.0)

        # x prefetch for chunk 0
        nc.sync.dma_start(
            out=t_rhs[0][98:128, :], in_=xt_in[:, 0:TC, :].rearrange("c t b -> c (t b)")
        )

        def th_col(c):
            return 0 if c == 0 else (1 if c == 1 else 2)

        for c in range(N_CHUNKS + 2):
            cur = t_rhs[c % 2]
            nxt = t_rhs[(c + 1) % 2]
            ps = t_ps[c % 2]
            th = t_th[:, th_col(c):th_col(c) + 1]

            # prefetch x for chunk c+1 (overlaps this chunk's scan)
            if c + 1 < N_CHUNKS:
                nc.sync.dma_start(
                    out=nxt[98:128, :],
                    in_=xt_in[:, (c + 1) * TC:(c + 2) * TC, :].rearrange(
                        "c t b -> c (t b)"),
                )

            # stacked matmul for this chunk, 4 sub-matmuls of 512 columns,
            # each copied PSUM->SBUF by the (otherwise idle) ACT engine
            hs = t_hs[c % 2]
            for blk in range(4):
                sl = slice(blk * 512, (blk + 1) * 512)
                nc.tensor.matmul(ps[:, sl], t_lhsT[:], cur[0:128, sl])
                nc.scalar.copy(hs[:, sl], ps[:, sl])

            # serial LIF scan. Columns [0:SP) run on DVE, columns [SP:B) on
            # the Pool engine -- each engine software-pipelines two column
            # sub-slices so every same-engine sem hop is covered by the other
            # slice's op. Spike ops all run on Pool.
            for tau in range(TC):
                if tau == 0:
                    r_ap = cur[0:98, (TC - 1) * B:TC * B]
                else:
                    r_ap = nxt[0:98, (tau - 1) * B:tau * B]
                h0 = tau * B

                def stt(eng, lo, hi):
                    eng.scalar_tensor_tensor(
                        t_u[:, lo:hi], t_M[:, lo:hi], BETA,
                        hs[:, h0 + lo:h0 + hi], AOT.mult, AOT.add)

                def tt(eng, lo, hi):
                    eng.tensor_tensor(
                        t_M[:, lo:hi], t_u[:, lo:hi], r_ap[:, lo:hi],
                        AOT.subtract)

                def spike(eng, lo, hi):
                    eng.tensor_scalar(
                        nxt[0:98, tau * B + lo:tau * B + hi],
                        t_M[:, lo:hi], th, None, AOT.is_gt)

                HA = SP // 2
                stt(nc.vector, 0, HA)
                stt(nc.vector, HA, SP)
                if SP < B:
                    PH = (B - SP) // 2
                    stt(nc.gpsimd, SP, SP + PH)
                    stt(nc.gpsimd, SP + PH, B)
                tt(nc.vector, 0, HA)
                tt(nc.vector, HA, SP)
                if SP < B:
                    tt(nc.gpsimd, SP, SP + PH)
                    tt(nc.gpsimd, SP + PH, B)
                # spike s(tau) = (M > th) -> nxt col tau (doubles as the
                # boundary column when tau == TC-1)
                spike(nc.gpsimd, 0, HA)
                spike(nc.gpsimd, HA, SP)
                if SP < B:
                    spike(nc.gpsimd, SP, B)

            # stash layer-3 spike rows for frame f = c-2
            f = c - 2
            if 0 <= f < N_CHUNKS:
                p0 = (f % 16) * TC
                cb = f // 16
                for j in range(2):
                    nc.sync.dma_start(
                        out=t_stash[p0:p0 + TC,
                                    cb * 512 + j * B:cb * 512 + (j + 1) * B],
                        in_=nxt[96 + j:97 + j, :],
                    )

        # reduce stash: sum the 8 column blocks, then sum over partitions
        nc.vector.tensor_tensor(
            t_part[:], t_stash[:, 0:512], t_stash[:, 512:1024], AOT.add)
        for cb in range(2, 8):
            nc.vector.tensor_tensor(
                t_part[:], t_part[:], t_stash[:, cb * 512:(cb + 1) * 512], AOT.add)
        nc.tensor.matmul(t_psr[:], t_ones[:], t_part[:])
        nc.scalar.copy(t_accf[:], t_psr[:])
        nc.gpsimd.dma_start(out=acc_out[:], in_=t_accf[:])

    _split_multi_waits(nc)
    return nc


def _host_consts(W1, b1, W2, b2, W3, b3):
    lhsT = np.zeros((128, 98), np.float32)
    lhsT[98:128, 0:64] = W1.T
    lhsT[0:64, 64:96] = W2.T
    lhsT[64:96, 96:98] = W3.T
    c1 = (b1.astype(np.float64) / (1.0 - BETA)).astype(np.float32)
    c2 = (b2.astype(np.float64) / (1.0 - BETA)).astype(np.float32)
    c3 = (b3.astype(np.float64) / (1.0 - BETA)).astype(np.float32)
    th_main = np.concatenate([
        (1.0 - c1.astype(np.float64)).astype(np.float32),
        (1.0 - c2.astype(np.float64)).astype(np.float32),
        (1.0 - c3.astype(np.float64)).astype(np.float32),
    ]).astype(np.float32)
    th_w0 = th_main.copy()
    th_w0[64:98] = BIG
    th_w1 = th_main.copy()
    th_w1[96:98] = BIG
    th = np.stack([th_w0, th_w1, th_main], axis=1)  # [98, 3]
    beta64 = np.float64(np.float32(BETA))
    m0 = np.zeros(98, np.float32)
    m0[0:64] = -c1
    m0[64:96] = (-c2.astype(np.float64) / beta64 ** TC).astype(np.float32)
    m0[96:98] = (-c3.astype(np.float64) / beta64 ** (2 * TC)).astype(np.float32)
    m0b = np.ascontiguousarray(
        np.broadcast_to(m0[:, None], (98, B))).astype(np.float32)
    return lhsT, th, m0b


def kernel(x, W1, b1, W2, b2, W3, b3):
    from concourse.bass_utils import run_bass_kernel_spmd

    x = np.asarray(x, np.float32)
    W1 = np.asarray(W1, np.float32); b1 = np.asarray(b1, np.float32)
    W2 = np.asarray(W2, np.float32); b2 = np.asarray(b2, np.float32)
    W3 = np.asarray(W3, np.float32); b3 = np.asarray(b3, np.float32)

    if "nc" not in _cache:
        _cache["nc"] = _build_program()
    nc = _cache["nc"]

    lhsT, th, m0b = _host_consts(W1, b1, W2, b2, W3, b3)
    in_maps = []
    for core in range(N_CORES):
        xs = x[core * B:(core + 1) * B]                # [256, 30, 1024]
        xt = np.ascontiguousarray(np.transpose(xs, (1, 2, 0)))  # [30, 1024, 256]
        in_maps.append({"xt": xt, "lhsT": lhsT, "th": th, "m0": m0b})

    res = run_bass_kernel_spmd(nc, in_maps, list(range(N_CORES)))
    out = np.empty((B_FULL, 2), np.float32)
    for core in range(N_CORES):
        a = res.results[core]["acc"].reshape(2, 256)   # [j, b]
        out[core * B:(core + 1) * B] = a.T
    return out


# revision 16
# speedup vs baseline: 6384.4152x; 1.0042x over previous
"""Trainium2 Bass kernel for nn_BinarySquareClassifier (3-layer LIF SNN).

Strategy (pure data parallel over batch, 8 cores, B=2048 -> 256/core):
- One stacked f32 matmul per 8-step time-chunk computes h1/h2/h3 for all
  three layers at once: lhsT [128, 98] holds W2.T/W3.T against the spike
  rows (rhs rows 0:96) and W1.T against the x rows (rhs rows 98:128);
  layer l's inputs come one chunk later than layer l-1's outputs (pipeline
  skew), so the serial LIF scans of the three layers run on time-shifted
  frames and stack into one [98, 256] membrane state M.
- Biases are folded away with the shift m^ = m - b/(1-beta): per-partition
  spike thresholds th = 1 - b/(1-beta), zero-input warmup freezing via +BIG
  thresholds and pre-decayed initial states.
- Per scan step, software-pipelined over two batch halves on DVE
  (u = beta*M + H then M = u - r, the halves' ops interleaved to cover
  semaphore latency); the spike op r = (M > th) runs on the Pool engine
  and doubles as the spike row written into the next chunk's matmul rhs.
  The ACT engine copies each PSUM matmul block to SBUF for cheaper reads.
- Layer-3 spike rows (rhs rows 96:98) are stashed to a [128, 4096] SBUF
  tile via SBUF->SBUF DMA (partition = time) and reduced at the end with
  tensor adds + a ones-vector matmul over partitions.
"""

import numpy as np
from contextlib import ExitStack

B_FULL, C_IN, T_FULL = 2048, 30, 1024
N_CORES = 8
B = B_FULL // N_CORES           # 256 batch per core
TC = 8                          # timesteps per chunk
N_CHUNKS = T_FULL // TC         # 128
BETA = 0.9
BIG = 3.0e38

_cache = {}


def _split_multi_waits(nc):
    """This container's walrus accepts only ONE sync-wait per instruction;
    hoist extra waits onto same-engine NoOps inserted just before."""
    import concourse.mybir as mybir
    counter = 0
    for f in nc.m.functions:
        for blk in f.blocks:
            out = []
            changed = False
            for inst in blk.instructions:
                si = inst.sync_info
                if si is not None and si.on_wait is not None and len(si.on_wait) > 1:
                    waits = list(si.on_wait)
                    for w in waits[:-1]:
                        counter += 1
                        nop = mybir.InstNoOp(
                            name=f"waitsplit-{counter}", ins=[], outs=[])
                        nop.engine = inst.engine
                        nop.sync_info = mybir.SyncInfo(on_wait=[w], on_update=[])
                        out.append(nop)
                    inst.sync_info = mybir.SyncInfo(
                        on_wait=[waits[-1]], on_update=list(si.on_update or []))
                    changed = True
                out.append(inst)
            if changed:
                try:
                    blk.instructions[:] = out
                except TypeError:
                    blk.instructions = out


def _build_program(SP=B):
    import concourse.bass as bass
    import concourse.mybir as mybir
    import concourse.tile as tile

    nc = bass.Bass("TRN2", target_bir_lowering=False, debug=False,
                   num_devices=N_CORES)
    dt = mybir.dt.float32
    AOT = mybir.AluOpType

    xt_in = nc.dram_tensor("xt", [C_IN, T_FULL, B], dt, kind="ExternalInput").ap()
    lhsT_in = nc.dram_tensor("lhsT", [128, 98], dt, kind="ExternalInput").ap()
    th_in = nc.dram_tensor("th", [98, 3], dt, kind="ExternalInput").ap()
    m0_in = nc.dram_tensor("m0", [98, B], dt, kind="ExternalInput").ap()
    acc_out = nc.dram_tensor("acc", [1, 512], dt, kind="ExternalOutput").ap()

    with ExitStack() as ctx:
        tc = ctx.enter_context(tile.TileContext(nc))
        pool = ctx.enter_context(tc.tile_pool(name="sb", bufs=1))
        psum_pool = ctx.enter_context(tc.tile_pool(name="ps", bufs=1, space="PSUM"))

        t_lhsT = pool.tile([128, 98], dt, tag="lhsT", name="lhsT")
        t_th = pool.tile([98, 3], dt, tag="th", name="th")
        t_M = pool.tile([98, B], dt, tag="M", name="M")
        t_u = pool.tile([98, B], dt, tag="u", name="u")
        t_rhs = [pool.tile([128, TC * B], dt, tag=f"rhs{i}", name=f"rhs{i}") for i in range(2)]
        t_stash = pool.tile([128, 4096], dt, tag="stash", name="stash")
        t_ones = pool.tile([128, 1], dt, tag="ones", name="ones")
        t_part = pool.tile([128, 512], dt, tag="part", name="part")
        t_accf = pool.tile([1, 512], dt, tag="accf", name="accf")
        t_ps = [psum_pool.tile([98, TC * B], dt, tag=f"H{i}", name=f"H{i}") for i in range(2)]
        t_hs = [pool.tile([98, TC * B], dt, tag=f"Hs{i}", name=f"Hs{i}") for i in range(2)]
        t_psr = t_ps[1][0:1, 0:512]  # reuse a PSUM bank for the final reduce

        nc.gpsimd.dma_start(out=t_lhsT[:], in_=lhsT_in[:])
        nc.gpsimd.dma_start(out=t_th[:], in_=th_in[:])
        nc.gpsimd.dma_start(out=t_M[:], in_=m0_in[:])
        nc.gpsimd.memset(t_ones[:], 1.0)
        # zero the spike rows of both rhs buffers and the stash
        for r in t_rhs:
            nc.gpsimd.memset(r[:], 0.0)

        # x prefetch for chunk 0, split per matmul block so the first
        # sub-matmul starts as soon as its two t-columns have landed
        for blk in range(4):
            nc.sync.dma_start(
                out=t_rhs[0][98:128, blk * 512:(blk + 1) * 512],
                in_=xt_in[:, blk * 2:(blk + 1) * 2, :].rearrange(
                    "c t b -> c (t b)"),
            )

        def th_col(c):
            return 0 if c == 0 else (1 if c == 1 else 2)

        for c in range(N_CHUNKS + 2):
            cur = t_rhs[c % 2]
            nxt = t_rhs[(c + 1) % 2]
            ps = t_ps[c % 2]
            th = t_th[:, th_col(c):th_col(c) + 1]

            # prefetch x for chunk c+1 (overlaps this chunk's scan)
            if c + 1 < N_CHUNKS:
                nc.sync.dma_start(
                    out=nxt[98:128, :],
                    in_=xt_in[:, (c + 1) * TC:(c + 2) * TC, :].rearrange(
                        "c t b -> c (t b)"),
                )

            # stacked matmul for this chunk, 4 sub-matmuls of 512 columns,
            # each copied PSUM->SBUF by the (otherwise idle) ACT engine
            hs = t_hs[c % 2]
            for blk in range(4):
                sl = slice(blk * 512, (blk + 1) * 512)
                nc.tensor.matmul(ps[:, sl], t_lhsT[:], cur[0:128, sl])
                nc.scalar.copy(hs[:, sl], ps[:, sl])

            # serial LIF scan. Columns [0:SP) run on DVE, columns [SP:B) on
            # the Pool engine -- each engine software-pipelines two column
            # sub-slices so every same-engine sem hop is covered by the other
            # slice's op. Spike ops all run on Pool.
            for tau in range(TC):
                if tau == 0:
                    r_ap = cur[0:98, (TC - 1) * B:TC * B]
                else:
                    r_ap = nxt[0:98, (tau - 1) * B:tau * B]
                h0 = tau * B

                def stt(eng, lo, hi):
                    eng.scalar_tensor_tensor(
                        t_u[:, lo:hi], t_M[:, lo:hi], BETA,
                        hs[:, h0 + lo:h0 + hi], AOT.mult, AOT.add)

                def tt(eng, lo, hi):
                    eng.tensor_tensor(
                        t_M[:, lo:hi], t_u[:, lo:hi], r_ap[:, lo:hi],
                        AOT.subtract)

                def spike(eng, lo, hi):
                    eng.tensor_scalar(
                        nxt[0:98, tau * B + lo:tau * B + hi],
                        t_M[:, lo:hi], th, None, AOT.is_gt)

                HA = SP // 2
                stt(nc.vector, 0, HA)
                stt(nc.vector, HA, SP)
                if SP < B:
                    PH = (B - SP) // 2
                    stt(nc.gpsimd, SP, SP + PH)
                    stt(nc.gpsimd, SP + PH, B)
                tt(nc.vector, 0, HA)
                tt(nc.vector, HA, SP)
                if SP < B:
                    tt(nc.gpsimd, SP, SP + PH)
                    tt(nc.gpsimd, SP + PH, B)
                # spike s(tau) = (M > th) -> nxt col tau (doubles as the
                # boundary column when tau == TC-1)
                spike(nc.gpsimd, 0, HA)
                spike(nc.gpsimd, HA, SP)
                if SP < B:
                    spike(nc.gpsimd, SP, B)

            # stash layer-3 spike rows for frame f = c-2
            f = c - 2
            if 0 <= f < N_CHUNKS:
                p0 = (f % 16) * TC
                cb = f // 16
                for j in range(2):
                    nc.sync.dma_start(
                        out=t_stash[p0:p0 + TC,
                                    cb * 512 + j * B:cb * 512 + (j + 1) * B],
                        in_=nxt[96 + j:97 + j, :],
                    )

        # reduce stash: sum the 8 column blocks, then sum over partitions
        nc.vector.tensor_tensor(
            t_part[:], t_stash[:, 0:512], t_stash[:, 512:1024], AOT.add)
        for cb in range(2, 8):
            nc.vector.tensor_tensor(
                t_part[:], t_part[:], t_stash[:, cb * 512:(cb + 1) * 512], AOT.add)
        nc.tensor.matmul(t_psr[:], t_ones[:], t_part[:])
        nc.scalar.copy(t_accf[:], t_psr[:])
        nc.gpsimd.dma_start(out=acc_out[:], in_=t_accf[:])

    _split_multi_waits(nc)
    return nc


def _host_consts(W1, b1, W2, b2, W3, b3):
    lhsT = np.zeros((128, 98), np.float32)
    lhsT[98:128, 0:64] = W1.T
    lhsT[0:64, 64:96] = W2.T
    lhsT[64:96, 96:98] = W3.T
    c1 = (b1.astype(np.float64) / (1.0 - BETA)).astype(np.float32)
    c2 = (b2.astype(np.float64) / (1.0 - BETA)).astype(np.float32)
    c3 = (b3.astype(np.float64) / (1.0 - BETA)).astype(np.float32)
    th_main = np.concatenate([
        (1.0 - c1.astype(np.float64)).astype(np.float32),
        (1.0 - c2.astype(np.float64)).astype(np.float32),
        (1.0 - c3.astype(np.float64)).astype(np.float32),
    ]).astype(np.float32)
    th_w0 = th_main.copy()
    th_w0[64:98] = BIG
    th_w1 = th_main.copy()
    th_w1[96:98] = BIG
    th = np.stack([th_w0, th_w1, th_main], axis=1)  # [98, 3]
    beta64 = np.float64(np.float32(BETA))
    m0 = np.zeros(98, np.float32)
    m0[0:64] = -c1
    m0[64:96] = (-c2.astype(np.float64) / beta64 ** TC).astype(np.float32)
    m0[96:98] = (-c3.astype(np.float64) / beta64 ** (2 * TC)).astype(np.float32)
    m0b = np.ascontiguousarray(
        np.broadcast_to(m0[:, None], (98, B))).astype(np.float32)
    return lhsT, th, m0b


def kernel(x, W1, b1, W2, b2, W3, b3):
    from concourse.bass_utils import run_bass_kernel_spmd

    x = np.asarray(x, np.float32)
    W1 = np.asarray(W1, np.float32); b1 = np.asarray(b1, np.float32)
    W2 = np.asarray(W2, np.float32); b2 = np.asarray(b2, np.float32)
    W3 = np.asarray(W3, np.float32); b3 = np.asarray(b3, np.float32)

    if "nc" not in _cache:
        _cache["nc"] = _build_program()
    nc = _cache["nc"]

    lhsT, th, m0b = _host_consts(W1, b1, W2, b2, W3, b3)
    in_maps = []
    for core in range(N_CORES):
        xs = x[core * B:(core + 1) * B]                # [256, 30, 1024]
        xt = np.ascontiguousarray(np.transpose(xs, (1, 2, 0)))  # [30, 1024, 256]
        in_maps.append({"xt": xt, "lhsT": lhsT, "th": th, "m0": m0b})

    res = run_bass_kernel_spmd(nc, in_maps, list(range(N_CORES)))
    out = np.empty((B_FULL, 2), np.float32)
    for core in range(N_CORES):
        a = res.results[core]["acc"].reshape(2, 256)   # [j, b]
        out[core * B:(core + 1) * B] = a.T
    return out
